# revision 30
# baseline (speedup 1.0000x reference)
"""Trainium2 Bass kernel for nn_SamplingBlock (gnn_message_passing).

Strategy
--------
8 cores = (batch b in 0..3) x (vertex half h in 0..1); each core owns 4096
vertices of one batch, fully data-parallel (no collectives).

Host-side weight folding (weights-only algebra):
    M_k   = W_sum[:,:,k] @ W_diff          (k = 0..8; [256, 259])
    M_0  += W_center
    bias  = sum_k W_sum[:,:,k] @ b_diff + b_sum + b_center       ([256])
    out[n] = sum_k M_k @ [feat_k; coords_k; 1]  (the 1-row carries the bias)

Host-side volume re-layout (fp16 "block table"):
    For every voxel r = z*1024 + y*32 + x the table stores the full 2x2x2
    neighbourhood as one contiguous 4 KB element of 8 rows x 256 ch:
      rows 0..3:  lo_zy  = vol[z+dz, y+dy, x]        (zy = dz*2+dy)
      rows 4..7:  d_zy   = vol[z+dz, y+dy, x+1] - lo (x+1 clamped)
    One dma_gather per 512-point sample fetches everything trilinear needs.

Device pipeline per 512-pt sample (Tile framework):
  1. x-lerp on DVE at packed rates: dm = d*fx (tensor_scalar, 4x mode)
     + lo (tensor_tensor, 2x mode)  -> [128 pts, 4 zy, 256 ch] fp16
  2. y/z-lerp folded into the PE transposes: 8 accumulating matmuls with
     rhs = diag(w_zy) produce the blended AND transposed features directly
     in PSUM:  featT[ch, pt] = sum_zy dm[pt, zy, ch] * w_zy[pt]
  3. main matmul: featT (fp16) x folded weights, PSUM accumulation over
     k = 0..8 (center + 8 neighbours) -> out [pts, 256]
Center samples additionally run the shift matmul -> neighbour coords ->
neighbour index math -> batched idx DMA round-trip -> 8 neighbour gathers.
Center gathers are prefetched one chunk ahead to keep DMA busy.
"""

import os
import sys

import numpy as np

for _p in ("/opt/trn_rl_repo", "/root/.axon_site/_ro/trn_rl_repo"):
    if os.path.isdir(_p) and _p not in sys.path:
        sys.path.insert(0, _p)
        break

import concourse.bacc as bacc
import concourse.bass as bass
import concourse.mybir as mybir
import concourse.tile as tile
from concourse.bass_utils import run_bass_kernel_spmd
from concourse.masks import make_identity

# ---------------------------------------------------------------- constants
B, N, C, NN = 4, 8192, 256, 8
GRID = 32
V = GRID * GRID * GRID             # 32768 rows
NVC = N // 2                       # vertices per core = 4096
VCHUNK = 512                       # vertices per chunk
GPC = VCHUNK // 128                # 128-pt groups per chunk = 4
ES = 8 * C                         # gather element: 8 rows x 256 ch (fp16)
F32 = mybir.dt.float32
F16 = mybir.dt.float16
I16 = mybir.dt.int16
I32 = mybir.dt.int32
ALU = mybir.AluOpType


# ------------------------------------------------------------- device program
def _emit_index_math(nc, sb, coords, S, r16_out, frc, w4, pfx):
    """coords: [128, S, 3] f32 AP (x, y, z normalized, unclipped).
    Writes r16_out [128, S] int16 row indices, frc [128, S, 3] f32
    fractions (frc[..,0] = fx) and w4 [128, S, 4] f32 zy corner weights
    ordered j = dz*2 + dy."""
    g = sb.tile([128, S, 3], F32, tag=pfx + "ixg")
    nc.vector.tensor_scalar(g[:], coords, 15.5, 15.5, op0=ALU.mult, op1=ALU.add)
    nc.vector.tensor_scalar(g[:], g[:], float(GRID - 1), 0.0, op0=ALU.min,
                            op1=ALU.max)
    # floor(g) robust to f32->int rounding mode: q = int(g); q -= (g < q)
    qi = sb.tile([128, S, 3], I32, tag=pfx + "ixq")
    nc.vector.tensor_copy(qi[:], g[:])
    i0 = sb.tile([128, S, 3], F32, tag=pfx + "ixi")
    nc.vector.tensor_copy(i0[:], qi[:])
    nc.vector.tensor_tensor(frc[:], g[:], i0[:], op=ALU.subtract)
    msk = sb.tile([128, S, 3], F32, tag=pfx + "ixm")
    nc.vector.tensor_scalar(msk[:], frc[:], 0.0, None, op0=ALU.is_lt)
    nc.vector.tensor_tensor(i0[:], i0[:], msk[:], op=ALU.subtract)
    nc.vector.tensor_tensor(frc[:], g[:], i0[:], op=ALU.subtract)
    # r00 = z*1024 + y*32 + x  (exact in f32)
    r = sb.tile([128, S], F32, tag=pfx + "ixr")
    nc.vector.tensor_scalar(r[:], i0[:, :, 2], 1024.0, None, op0=ALU.mult)
    t = sb.tile([128, S], F32, tag=pfx + "ixt")
    nc.vector.tensor_scalar(t[:], i0[:, :, 1], 32.0, None, op0=ALU.mult)
    nc.vector.tensor_tensor(r[:], r[:], t[:], op=ALU.add)
    nc.vector.tensor_tensor(r[:], r[:], i0[:, :, 0], op=ALU.add)
    nc.vector.tensor_copy(r16_out, r[:])
    inv = sb.tile([128, S, 3], F32, tag=pfx + "ixv")
    nc.vector.tensor_scalar(inv[:], frc[:], -1.0, 1.0, op0=ALU.mult, op1=ALU.add)
    # w4[j = dz*2+dy]: (dy ? fy : 1-fy) * (dz ? fz : 1-fz)
    for j, (ys, zs) in enumerate(((inv, inv), (frc, inv), (inv, frc),
                                  (frc, frc))):
        nc.vector.tensor_tensor(w4[:, :, j], ys[:, :, 1], zs[:, :, 2],
                                op=ALU.mult)


def build_program(nvc=NVC):
    nchunk = nvc // VCHUNK
    nc = bacc.Bacc("TRN2", target_bir_lowering=False, debug=False)

    verts_d = nc.dram_tensor("verts", [nvc, 3], F32, kind="ExternalInput")
    table_d = nc.dram_tensor("table", [V * ES], F16, kind="ExternalInput")
    msum_a_d = nc.dram_tensor("msum_a", [128, 9, C], F16, kind="ExternalInput")
    msum_b_d = nc.dram_tensor("msum_b", [128, 9, C], F16, kind="ExternalInput")
    msum_c_d = nc.dram_tensor("msum_c", [36, C], F16, kind="ExternalInput")
    wsh_a_d = nc.dram_tensor("wsh_a", [128, 3 * NN], F16, kind="ExternalInput")
    wsh_b_d = nc.dram_tensor("wsh_b", [128, 3 * NN], F16, kind="ExternalInput")
    bsh_d = nc.dram_tensor("bsh", [128, NN, 3], F32, kind="ExternalInput")
    rep16_d = nc.dram_tensor("rep16", [16, 128], F32, kind="ExternalInput")
    out_d = nc.dram_tensor("out", [nvc, C], F32, kind="ExternalOutput")

    tbl_ap = bass.AP(table_d, 0, [[ES, V], [1, ES]])
    SC = nvc // 128                     # center cols per partition

    with tile.TileContext(nc) as tc:
        with (
            tc.tile_pool(name="cst", bufs=1) as cst,
            tc.tile_pool(name="wp", bufs=1) as wp,
            tc.tile_pool(name="ix", bufs=3) as ixp,
            tc.tile_pool(name="cg", bufs=3) as cgp,
            tc.tile_pool(name="ng", bufs=2) as gp,
            tc.tile_pool(name="xl", bufs=3) as xp,
            tc.tile_pool(name="dg", bufs=2) as dgp,
            tc.tile_pool(name="ft", bufs=3) as ftp,
            tc.tile_pool(name="mi", bufs=2) as mp,
            tc.tile_pool(name="drc", bufs=1, space="DRAM") as dpc,
            tc.tile_pool(name="drn", bufs=3, space="DRAM") as dpn,
            tc.tile_pool(name="pso", bufs=1, space="PSUM") as pso,
            tc.tile_pool(name="psf", bufs=2, space="PSUM") as psf,
            tc.tile_pool(name="pss", bufs=1, space="PSUM") as pss,
            tc.tile_pool(name="pst", bufs=1, space="PSUM") as pst,
        ):
            ident = cst.tile([128, 128], F16)
            make_identity(nc, ident[:])
            msum_a = cst.tile([128, 9, C], F16)
            msum_b = cst.tile([128, 9, C], F16)
            msum_c = cst.tile([36, C], F16)
            wsh_a = cst.tile([128, 3 * NN], F16)
            wsh_b = cst.tile([128, 3 * NN], F16)
            bsh = cst.tile([128, NN, 3], F32)
            rep16 = cst.tile([16, 128], F32)
            nc.sync.dma_start(msum_a[:], msum_a_d[:])
            nc.sync.dma_start(msum_b[:], msum_b_d[:])
            nc.sync.dma_start(msum_c[:], msum_c_d[:])
            nc.sync.dma_start(wsh_a[:], wsh_a_d[:])
            nc.sync.dma_start(wsh_b[:], wsh_b_d[:])
            nc.sync.dma_start(bsh[:], bsh_d[:])
            nc.sync.dma_start(rep16[:], rep16_d[:])

            verts = cst.tile([128, SC, 3], F32)
            nc.sync.dma_start(
                verts[:], verts_d[:].rearrange("(vt p) c -> p vt c", p=128))

            # ---- whole-core center index math + coords4 ----
            r16c = wp.tile([128, SC], I16)
            frcC = wp.tile([128, SC, 3], F32)
            w4c = wp.tile([128, SC, 4], F16)
            _emit_index_math(nc, wp, verts[:], SC, r16c[:], frcC, w4c, "c")

            # center indices: DRAM round-trip into wrapped-16 layout, then
            # replicate to 128 partitions via the rep16 matmul -- once for
            # the whole core.
            scr_c = dpc.tile([nvc], I16)
            nc.sync.dma_start(
                scr_c[:].rearrange("(vt p) -> p vt", p=128), r16c[:])
            t16c = wp.tile([16, nvc // 16], I16)
            nc.sync.dma_start(
                t16c[:], scr_c[:].rearrange("(m q) -> q m", q=16))
            fc = wp.tile([16, nvc // 16], F32)
            nc.vector.tensor_copy(fc[:], t16c[:])
            idxc = wp.tile([128, nvc // 16], I16)
            for half in range(max(1, nvc // 4096)):
                lo, hi = half * 256, min((half + 1) * 256, nvc // 16)
                pr = psf.tile([128, 2, 128], F32, space="PSUM", tag="pF",
                              name=f"repc{half}")
                pr_v = pr[:].rearrange("p a b -> p (a b)")[:, 0:hi - lo]
                nc.tensor.matmul(pr_v, rep16[:], fc[:, lo:hi], start=True,
                                 stop=True)
                nc.vector.tensor_copy(idxc[:, lo:hi], pr_v)

            def gather512(idx_ap, pool, tag):
                gt = pool.tile([128, GPC, ES], F16, tag=tag)
                nc.gpsimd.dma_gather(gt[:], tbl_ap, idx_ap, VCHUNK, VCHUNK, ES)
                return gt

            def xlerp(gt, g, fx_ap):
                """[128 pts, 4 zy, 256 ch] = diff * fx + lo  (fp16, one op)."""
                dm = xp.tile([128, 4, C], F16, tag="dm")
                nc.vector.scalar_tensor_tensor(
                    dm[:].rearrange("p z c -> p (z c)"), gt[:, g, 1024:2048],
                    fx_ap, gt[:, g, 0:1024], op0=ALU.mult, op1=ALU.add)
                return dm

            def diag16(w4_ap, tag):
                """[128, GPC, 4, 128] diag tiles for a whole 512-pt sample:
                one broadcast multiply builds all 16 diagonals.
                w4_ap: [128, GPC, 4] fp16 zy-weights per point."""
                dg = dgp.tile([128, GPC, 4, 128], F16, tag=tag)
                nc.vector.tensor_tensor(
                    dg[:],
                    ident[:].rearrange("p (a b c) -> p a b c", a=1, b=1)
                    .to_broadcast([128, GPC, 4, 128]),
                    w4_ap.rearrange("p g (z u) -> p g z u", u=1)
                    .to_broadcast([128, GPC, 4, 128]),
                    op=ALU.mult)
                return dg

            def feat_transpose(dm, dg_g, tag="fsb", bufs=None):
                """8 accumulating diag-matmuls: blended featT in PSUM ->
                fp16 SBUF [128 ch-half, 2, 128 pts].
                dg_g: [128, 4, 128] diag tiles (one per zy) for this group."""
                pF = psf.tile([128, 2, 128], F32, space="PSUM", tag="pF")
                for h in range(2):
                    for zy in range(4):
                        nc.tensor.matmul(
                            pF[:, h, :], dm[:, zy, h * 128:(h + 1) * 128],
                            dg_g[:, zy, :], start=(zy == 0), stop=(zy == 3))
                fsb = ftp.tile([128, 2, 128], F16, tag=tag, bufs=bufs)
                nc.scalar.copy(fsb[:], pF[:])
                return fsb

            def feat_transpose16(gt, j, dgn_g, dgf_g):
                """PE-path blend: 16 accumulating diag-matmuls straight off
                the gathered tile (x-lerp folded in via the fx*w4 diags)."""
                pF = psf.tile([128, 2, 128], F32, space="PSUM", tag="pF")
                for hh in range(2):
                    for zy in range(4):
                        o = zy * 256 + hh * 128
                        nc.tensor.matmul(
                            pF[:, hh, :], gt[:, j, o:o + 128],
                            dgn_g[:, zy, :], start=(zy == 0), stop=False)
                        nc.tensor.matmul(
                            pF[:, hh, :], gt[:, j, 1024 + o:1024 + o + 128],
                            dgf_g[:, zy, :], start=False, stop=(zy == 3))
                fsb = ftp.tile([128, 2, 128], F16, tag="fsb")
                nc.scalar.copy(fsb[:], pF[:])
                return fsb

            def main_mm(out_ap, fsb, k, start, stop):
                nc.tensor.matmul(out_ap, fsb[:, 0, :], msum_a[:, k, :],
                                 start=start, stop=False)
                nc.tensor.matmul(out_ap, fsb[:, 1, :], msum_b[:, k, :],
                                 start=False, stop=stop)

            def centers_compute(vc):
                """Everything for chunk vc's centers except the matmuls that
                touch the output accumulator (deferred so this block can be
                emitted while the PREVIOUS chunk's neighbours are in flight):
                gather, x-lerp, featT, shift, neighbour coords, neighbour
                index math, idx round-trip."""
                h = {}
                gts_c = gather512(idxc[:, vc * 32:(vc + 1) * 32], cgp, "cgt")
                dgc = diag16(w4c[:, vc * GPC:(vc + 1) * GPC, :], "dgc")
                ncoord = ixp.tile([128, GPC, NN, 3], F32, tag="ncrd")
                h["fsb"] = []
                for g in range(GPC):
                    vt = vc * GPC + g
                    dm = xlerp(gts_c, g, frcC[:, vt, 0:1])
                    fsb = feat_transpose(dm, dgc[:, g], tag="cfsb", bufs=12)
                    h["fsb"].append(fsb)
                    # shift matmul -> [128 pts, 24]
                    pS = pss.tile([128, 3 * NN], F32, space="PSUM", tag="sh")
                    nc.tensor.matmul(pS[:], fsb[:, 0, :], wsh_a[:],
                                     start=True, stop=False)
                    nc.tensor.matmul(pS[:], fsb[:, 1, :], wsh_b[:],
                                     start=False, stop=True)
                    ssb = mp.tile([128, 3 * NN], F32, tag="ssb")
                    nc.scalar.copy(ssb[:], pS[:])
                    # neighbour coords: shift + b_shift + verts [128, NN, 3]
                    nc.vector.tensor_tensor(
                        ncoord[:, g, :, :],
                        ssb[:].rearrange("p (nn c) -> p nn c", c=3),
                        bsh[:], op=ALU.add)
                    nc.vector.tensor_tensor(
                        ncoord[:, g, :, :], ncoord[:, g, :, :],
                        verts[:, vt:vt + 1, :].to_broadcast([128, NN, 3]),
                        op=ALU.add)

                # ---- neighbour index math (whole chunk, S = 32) ----
                r16n = ixp.tile([128, GPC * NN], I16, tag="r16n")
                frcN = ixp.tile([128, GPC * NN, 3], F32, tag="frcN")
                w4n = ixp.tile([128, GPC * NN, 4], F16, tag="w4n")
                _emit_index_math(
                    nc, ixp, ncoord[:].rearrange("p g nn c -> p (g nn) c"),
                    GPC * NN, r16n[:], frcN, w4n, "n")
                h["frcN"], h["w4n"] = frcN, w4n
                # [coords; 1] for all 9 samples, grouped by g so that one
                # [36, 128] transpose + one K=36 matmul per group covers the
                # whole coord part of the contraction.
                ca4 = ixp.tile([128, GPC, 9, 4], F16, tag="ca4")
                nc.vector.tensor_copy(
                    ca4[:, :, 0, 0:3], verts[:, vc * GPC:(vc + 1) * GPC, :])
                nc.vector.tensor_copy(ca4[:, :, 1:9, 0:3], ncoord[:])
                nc.vector.memset(ca4[:, :, :, 3], 1.0)
                h["ct36"] = []
                for g in range(GPC):
                    pTn = pst.tile([36, 128], F16, space="PSUM", tag="pTn")
                    nc.tensor.transpose(
                        pTn[:], ca4[:, g, :, :].rearrange("p k c -> p (k c)"),
                        ident[:])
                    ct36 = mp.tile([36, 128], F16, tag="ct36", bufs=12)
                    nc.scalar.copy(ct36[:], pTn[:])
                    h["ct36"].append(ct36)

                # idx round-trip: DRAM layout "(g p nn)" keeps both DMAs
                # nn-contiguous; read back wrapped-16 + replicate once.
                scr_n = dpn.tile([VCHUNK * NN], I16, tag="scrn")
                nc.sync.dma_start(
                    scr_n[:].rearrange("(g p nn) -> p g nn", p=128, g=GPC),
                    r16n[:].rearrange("p (g nn) -> p g nn", nn=NN))
                t16n = ixp.tile([16, NN, VCHUNK // 16], I16, tag="t16n")
                nc.sync.dma_start(
                    t16n[:].rearrange("q nn (m1 m0) -> q nn m1 m0", m1=GPC),
                    scr_n[:].rearrange("(m1 m0 q nn) -> q nn m1 m0",
                                       m1=GPC, m0=8, q=16))
                h["t16n"] = t16n
                h["vc"] = vc
                return h

            def centers_finish(h):
                """idx conversion + replication; emitted a few neighbour
                blocks after centers_compute so the scratch round-trip
                latency never stalls the in-order DVE stream."""
                t16n = h["t16n"]
                fn = ixp.tile([16, NN, VCHUNK // 16], F32, tag="fn")
                nc.vector.tensor_copy(fn[:], t16n[:])
                pRn = psf.tile([128, 2, 128], F32, space="PSUM",
                               tag="pF", name=f"repn{h['vc']}")
                pRn_v = pRn[:].rearrange("p a b -> p (a b)")
                nc.tensor.matmul(
                    pRn_v, rep16[:], fn[:].rearrange("q nn m -> q (nn m)"),
                    start=True, stop=True)
                idxn = ixp.tile([128, NN, VCHUNK // 16], I16, tag="idxn")
                nc.vector.tensor_copy(
                    idxn[:].rearrange("p nn m -> p (nn m)"), pRn_v)
                h["idxn"] = idxn

            # 2-deep software pipeline: chunk vc's neighbour phase runs
            # while chunk vc+2's center phase (emitted inside it) covers the
            # idx-chain and gather latency.
            hs = {0: centers_compute(0)}
            if nchunk > 1:
                hs[1] = centers_compute(1)
            centers_finish(hs[0])
            if nchunk > 1:
                centers_finish(hs[1])
            for vc in range(nchunk):
                h = hs.pop(vc)
                # one full 2 KB bank per group: psum allows only one pending
                # accumulation group per bank ("zero region")
                out4 = pso.tile([128, GPC, 2 * C], F32, space="PSUM",
                                tag="out", name=f"out{vc}")
                out_ps = [out4[:, g, 0:C] for g in range(GPC)]
                # deferred center + coord matmuls for this chunk
                for g in range(GPC):
                    main_mm(out_ps[g], h["fsb"][g], 0, start=True, stop=False)
                    nc.tensor.matmul(out_ps[g], h["ct36"][g][:], msum_c[:],
                                     start=False, stop=False)
                idxn, frcN, w4n = h["idxn"], h["frcN"], h["w4n"]
                w4n_v = w4n[:].rearrange("p (g nn) z -> p g nn z", nn=NN)
                # fx * w4 for the PE-path samples (one small op per chunk)
                fw4n = ixp.tile([128, GPC * NN, 4], F16, tag="fw4n")
                nc.vector.tensor_tensor(
                    fw4n[:], w4n[:],
                    frcN[:, :, 0:1].to_broadcast([128, GPC * NN, 4]),
                    op=ALU.mult)
                fw4n_v = fw4n[:].rearrange("p (g nn) z -> p g nn z", nn=NN)

                # ---- neighbours: gathers paired (1024 idx) to halve the
                # SWDGE fixed prep cost ----
                for pr in range(NN // 2):
                    gtn = gp.tile([128, 2 * GPC, ES], F16, tag="ngt")
                    nc.gpsimd.dma_gather(
                        gtn[:], tbl_ap, idxn[:, 2 * pr:2 * pr + 2, :],
                        2 * VCHUNK, 2 * VCHUNK, ES)
                    for sub in range(2):
                        nn_i = 2 * pr + sub
                        dgn = diag16(w4n_v[:, :, nn_i, :], "dgn")
                        if nn_i in (1, 4, 7):
                            # PE-path: x-lerp folds into 8 extra diag-mms
                            dgf = diag16(fw4n_v[:, :, nn_i, :], "dgf")
                            for g in range(GPC):
                                fsb = feat_transpose16(
                                    gtn, 4 * sub + g, dgn[:, g], dgf[:, g])
                                main_mm(out_ps[g], fsb, nn_i + 1, start=False,
                                        stop=(nn_i == NN - 1))
                        else:
                            for g in range(GPC):
                                col = g * NN + nn_i
                                dm = xlerp(gtn, 4 * sub + g, frcN[:, col, 0:1])
                                fsb = feat_transpose(dm, dgn[:, g])
                                main_mm(out_ps[g], fsb, nn_i + 1, start=False,
                                        stop=(nn_i == NN - 1))
                    if pr == 0 and vc + 2 < nchunk:
                        hs[vc + 2] = centers_compute(vc + 2)
                    if pr == 1 and vc + 2 < nchunk:
                        centers_finish(hs[vc + 2])

                # ---- epilogue ----
                for g in range(GPC):
                    osb = mp.tile([128, C], F32, tag="osb")
                    nc.scalar.copy(osb[:], out_ps[g])
                    lo = (vc * GPC + g) * 128
                    nc.sync.dma_start(out_d[lo:lo + 128, :], osb[:])

    nc.compile()
    return nc


# --------------------------------------------------------------- host wrapper
_CACHED = {}


def _block_index():
    """Static [V] row indices for the 8 block entries (zy lo + x+1)."""
    if "bidx" in _CACHED:
        return _CACHED["bidx"]
    z, y, x = np.meshgrid(np.arange(GRID), np.arange(GRID), np.arange(GRID),
                          indexing="ij")
    x1 = np.minimum(x + 1, GRID - 1)
    lo, hi = [], []
    for dz in (0, 1):
        for dy in (0, 1):
            zc = np.minimum(z + dz, GRID - 1)
            yc = np.minimum(y + dy, GRID - 1)
            lo.append(((zc * GRID + yc) * GRID + x).ravel())
            hi.append(((zc * GRID + yc) * GRID + x1).ravel())
    bidx = (np.stack(lo, 1).astype(np.int32), np.stack(hi, 1).astype(np.int32))
    _CACHED["bidx"] = bidx
    return bidx


def _host_prep(x, W_shift, b_shift, W_diff, b_diff, W_center, b_center,
               W_sum, b_sum):
    lo_i, hi_i = _block_index()
    tables = np.empty((B, V, 8, C), np.float16)
    for b in range(B):
        xt = np.ascontiguousarray(x[b].reshape(C, V).T)     # [V, C] f32
        lo = xt[lo_i]                                        # [V, 4, C]
        tables[b, :, 0:4, :] = lo
        tables[b, :, 4:8, :] = xt[hi_i] - lo
    tables = tables.reshape(B, V * ES)

    M = np.einsum("ock,cd->okd", W_sum.astype(np.float64),
                  W_diff.astype(np.float64))                 # [256, 9, 259]
    M = np.transpose(M, (1, 0, 2)).copy()                    # [9, 256, 259]
    M[0] += W_center.astype(np.float64)
    bias = (W_sum.astype(np.float64).sum(-1) @ b_diff.astype(np.float64)
            + b_sum + b_center)                              # [256]
    msum = np.zeros((9, C + 4, C), np.float16)
    for k in range(9):
        msum[k, :C + 3, :] = M[k].T.astype(np.float16)
    msum[0, C + 3, :] = bias.astype(np.float16)
    msum_a = np.ascontiguousarray(np.transpose(msum[:, 0:128, :], (1, 0, 2)))
    msum_b = np.ascontiguousarray(np.transpose(msum[:, 128:256, :], (1, 0, 2)))
    # [36, 256]: row k*4+j = coord row j (x, y, z, bias) of M_k
    msum_c = np.ascontiguousarray(
        msum[:, 256:260, :].reshape(36, C))

    wsh = W_shift.T.astype(np.float16)                       # [256, 24]
    bsh = np.broadcast_to(
        b_shift.astype(np.float32).reshape(NN, 3), (128, NN, 3)).copy()
    return (tables, msum_a, msum_b, msum_c,
            np.ascontiguousarray(wsh[0:128]),
            np.ascontiguousarray(wsh[128:256]), bsh)


def kernel(x, vertices, W_shift, b_shift, W_diff, b_diff, W_center, b_center,
           W_sum, b_sum):
    if "nc" not in _CACHED:
        _CACHED["nc"] = build_program()
    nc = _CACHED["nc"]

    tables, msum_a, msum_b, msum_c, wsh_a, wsh_b, bsh = _host_prep(
        x, W_shift, b_shift, W_diff, b_diff, W_center, b_center, W_sum, b_sum)

    in_maps = []
    for core in range(8):
        b, h = divmod(core, 2)
        in_maps.append({
            "verts": np.ascontiguousarray(
                vertices[b, h * NVC:(h + 1) * NVC]).astype(np.float32),
            "table": tables[b],
            "msum_a": msum_a, "msum_b": msum_b, "msum_c": msum_c,
            "wsh_a": wsh_a, "wsh_b": wsh_b, "bsh": bsh,
            "rep16": np.tile(np.eye(16, dtype=np.float32), 8),
        })

    res = run_bass_kernel_spmd(nc, in_maps, core_ids=list(range(8)))
    out = np.empty((B, N, C), np.float32)
    for core in range(8):
        b, h = divmod(core, 2)
        out[b, h * NVC:(h + 1) * NVC] = res.results[core]["out"]
    return out


# revision 32
# speedup vs baseline: 1.0033x; 1.0033x over previous
"""Trainium2 Bass kernel for nn_SamplingBlock (gnn_message_passing).

Strategy
--------
8 cores = (batch b in 0..3) x (vertex half h in 0..1); each core owns 4096
vertices of one batch, fully data-parallel (no collectives).

Host-side weight folding (weights-only algebra):
    M_k   = W_sum[:,:,k] @ W_diff          (k = 0..8; [256, 259])
    M_0  += W_center
    bias  = sum_k W_sum[:,:,k] @ b_diff + b_sum + b_center       ([256])
    out[n] = sum_k M_k @ [feat_k; coords_k; 1]  (the 1-row carries the bias)

Host-side volume re-layout (fp16 "block table"):
    For every voxel r = z*1024 + y*32 + x the table stores the full 2x2x2
    neighbourhood as one contiguous 4 KB element of 8 rows x 256 ch:
      rows 0..3:  lo_zy  = vol[z+dz, y+dy, x]        (zy = dz*2+dy)
      rows 4..7:  d_zy   = vol[z+dz, y+dy, x+1] - lo (x+1 clamped)
    One dma_gather per 512-point sample fetches everything trilinear needs.

Device pipeline per 512-pt sample (Tile framework):
  1. x-lerp on DVE at packed rates: dm = d*fx (tensor_scalar, 4x mode)
     + lo (tensor_tensor, 2x mode)  -> [128 pts, 4 zy, 256 ch] fp16
  2. y/z-lerp folded into the PE transposes: 8 accumulating matmuls with
     rhs = diag(w_zy) produce the blended AND transposed features directly
     in PSUM:  featT[ch, pt] = sum_zy dm[pt, zy, ch] * w_zy[pt]
  3. main matmul: featT (fp16) x folded weights, PSUM accumulation over
     k = 0..8 (center + 8 neighbours) -> out [pts, 256]
Center samples additionally run the shift matmul -> neighbour coords ->
neighbour index math -> batched idx DMA round-trip -> 8 neighbour gathers.
Center gathers are prefetched one chunk ahead to keep DMA busy.
"""

import os
import sys

import numpy as np

for _p in ("/opt/trn_rl_repo", "/root/.axon_site/_ro/trn_rl_repo"):
    if os.path.isdir(_p) and _p not in sys.path:
        sys.path.insert(0, _p)
        break

import concourse.bacc as bacc
import concourse.bass as bass
import concourse.mybir as mybir
import concourse.tile as tile
from concourse.bass_utils import run_bass_kernel_spmd
from concourse.masks import make_identity

# ---------------------------------------------------------------- constants
B, N, C, NN = 4, 8192, 256, 8
GRID = 32
V = GRID * GRID * GRID             # 32768 rows
NVC = N // 2                       # vertices per core = 4096
VCHUNK = 512                       # vertices per chunk
GPC = VCHUNK // 128                # 128-pt groups per chunk = 4
ES = 8 * C                         # gather element: 8 rows x 256 ch (fp16)
F32 = mybir.dt.float32
F16 = mybir.dt.float16
I16 = mybir.dt.int16
I32 = mybir.dt.int32
ALU = mybir.AluOpType


# ------------------------------------------------------------- device program
def _emit_index_math(nc, sb, coords, S, r16_out, frc, w4, pfx):
    """coords: [128, S, 3] f32 AP (x, y, z normalized, unclipped).
    Writes r16_out [128, S] int16 row indices, frc [128, S, 3] f32
    fractions (frc[..,0] = fx) and w4 [128, S, 4] f32 zy corner weights
    ordered j = dz*2 + dy."""
    g = sb.tile([128, S, 3], F32, tag=pfx + "ixg")
    nc.vector.tensor_scalar(g[:], coords, 15.5, 15.5, op0=ALU.mult, op1=ALU.add)
    nc.vector.tensor_scalar(g[:], g[:], float(GRID - 1), 0.0, op0=ALU.min,
                            op1=ALU.max)
    # floor(g) robust to f32->int rounding mode: q = int(g); q -= (g < q)
    qi = sb.tile([128, S, 3], I32, tag=pfx + "ixq")
    nc.vector.tensor_copy(qi[:], g[:])
    i0 = sb.tile([128, S, 3], F32, tag=pfx + "ixi")
    nc.vector.tensor_copy(i0[:], qi[:])
    nc.vector.tensor_tensor(frc[:], g[:], i0[:], op=ALU.subtract)
    msk = sb.tile([128, S, 3], F32, tag=pfx + "ixm")
    nc.vector.tensor_scalar(msk[:], frc[:], 0.0, None, op0=ALU.is_lt)
    nc.vector.tensor_tensor(i0[:], i0[:], msk[:], op=ALU.subtract)
    nc.vector.tensor_tensor(frc[:], g[:], i0[:], op=ALU.subtract)
    # r00 = z*1024 + y*32 + x  (exact in f32)
    r = sb.tile([128, S], F32, tag=pfx + "ixr")
    nc.vector.tensor_scalar(r[:], i0[:, :, 2], 1024.0, None, op0=ALU.mult)
    t = sb.tile([128, S], F32, tag=pfx + "ixt")
    nc.vector.tensor_scalar(t[:], i0[:, :, 1], 32.0, None, op0=ALU.mult)
    nc.vector.tensor_tensor(r[:], r[:], t[:], op=ALU.add)
    nc.vector.tensor_tensor(r[:], r[:], i0[:, :, 0], op=ALU.add)
    nc.vector.tensor_copy(r16_out, r[:])
    inv = sb.tile([128, S, 3], F32, tag=pfx + "ixv")
    nc.vector.tensor_scalar(inv[:], frc[:], -1.0, 1.0, op0=ALU.mult, op1=ALU.add)
    # w4[j = dz*2+dy]: (dy ? fy : 1-fy) * (dz ? fz : 1-fz)
    for j, (ys, zs) in enumerate(((inv, inv), (frc, inv), (inv, frc),
                                  (frc, frc))):
        nc.vector.tensor_tensor(w4[:, :, j], ys[:, :, 1], zs[:, :, 2],
                                op=ALU.mult)


def build_program(nvc=NVC):
    nchunk = nvc // VCHUNK
    nc = bacc.Bacc("TRN2", target_bir_lowering=False, debug=False)

    verts_d = nc.dram_tensor("verts", [nvc, 3], F32, kind="ExternalInput")
    table_d = nc.dram_tensor("table", [V * ES], F16, kind="ExternalInput")
    msum_a_d = nc.dram_tensor("msum_a", [128, 9, C], F16, kind="ExternalInput")
    msum_b_d = nc.dram_tensor("msum_b", [128, 9, C], F16, kind="ExternalInput")
    msum_c_d = nc.dram_tensor("msum_c", [36, C], F16, kind="ExternalInput")
    wsh_a_d = nc.dram_tensor("wsh_a", [128, 3 * NN], F16, kind="ExternalInput")
    wsh_b_d = nc.dram_tensor("wsh_b", [128, 3 * NN], F16, kind="ExternalInput")
    bsh_d = nc.dram_tensor("bsh", [128, NN, 3], F32, kind="ExternalInput")
    rep16_d = nc.dram_tensor("rep16", [16, 128], F32, kind="ExternalInput")
    out_d = nc.dram_tensor("out", [nvc, C], F32, kind="ExternalOutput")

    tbl_ap = bass.AP(table_d, 0, [[ES, V], [1, ES]])
    SC = nvc // 128                     # center cols per partition

    with tile.TileContext(nc) as tc:
        with (
            tc.tile_pool(name="cst", bufs=1) as cst,
            tc.tile_pool(name="wp", bufs=1) as wp,
            tc.tile_pool(name="ix", bufs=3) as ixp,
            tc.tile_pool(name="cg", bufs=3) as cgp,
            tc.tile_pool(name="ng", bufs=2) as gp,
            tc.tile_pool(name="xl", bufs=3) as xp,
            tc.tile_pool(name="dg", bufs=2) as dgp,
            tc.tile_pool(name="ft", bufs=3) as ftp,
            tc.tile_pool(name="mi", bufs=2) as mp,
            tc.tile_pool(name="drc", bufs=1, space="DRAM") as dpc,
            tc.tile_pool(name="drn", bufs=3, space="DRAM") as dpn,
            tc.tile_pool(name="pso", bufs=1, space="PSUM") as pso,
            tc.tile_pool(name="psf", bufs=2, space="PSUM") as psf,
            tc.tile_pool(name="pss", bufs=1, space="PSUM") as pss,
            tc.tile_pool(name="pst", bufs=1, space="PSUM") as pst,
        ):
            ident = cst.tile([128, 128], F16)
            make_identity(nc, ident[:])
            msum_a = cst.tile([128, 9, C], F16)
            msum_b = cst.tile([128, 9, C], F16)
            msum_c = cst.tile([36, C], F16)
            wsh_a = cst.tile([128, 3 * NN], F16)
            wsh_b = cst.tile([128, 3 * NN], F16)
            bsh = cst.tile([128, NN, 3], F32)
            rep16 = cst.tile([16, 128], F32)
            nc.sync.dma_start(msum_a[:], msum_a_d[:])
            nc.sync.dma_start(msum_b[:], msum_b_d[:])
            nc.sync.dma_start(msum_c[:], msum_c_d[:])
            nc.sync.dma_start(wsh_a[:], wsh_a_d[:])
            nc.sync.dma_start(wsh_b[:], wsh_b_d[:])
            nc.sync.dma_start(bsh[:], bsh_d[:])
            nc.sync.dma_start(rep16[:], rep16_d[:])

            verts = cst.tile([128, SC, 3], F32)
            nc.sync.dma_start(
                verts[:], verts_d[:].rearrange("(vt p) c -> p vt c", p=128))

            # ---- whole-core center index math + coords4 ----
            r16c = wp.tile([128, SC], I16)
            frcC = wp.tile([128, SC, 3], F32)
            w4c = wp.tile([128, SC, 4], F16)
            _emit_index_math(nc, wp, verts[:], SC, r16c[:], frcC, w4c, "c")

            # center indices: DRAM round-trip into wrapped-16 layout, then
            # replicate to 128 partitions via the rep16 matmul -- once for
            # the whole core.
            scr_c = dpc.tile([nvc], I16)
            nc.sync.dma_start(
                scr_c[:].rearrange("(vt p) -> p vt", p=128), r16c[:])
            t16c = wp.tile([16, nvc // 16], I16)
            nc.sync.dma_start(
                t16c[:], scr_c[:].rearrange("(m q) -> q m", q=16))
            fc = wp.tile([16, nvc // 16], F32)
            nc.vector.tensor_copy(fc[:], t16c[:])
            idxc = wp.tile([128, nvc // 16], I16)
            for half in range(max(1, nvc // 4096)):
                lo, hi = half * 256, min((half + 1) * 256, nvc // 16)
                pr = psf.tile([128, 2, 128], F32, space="PSUM", tag="pF",
                              name=f"repc{half}")
                pr_v = pr[:].rearrange("p a b -> p (a b)")[:, 0:hi - lo]
                nc.tensor.matmul(pr_v, rep16[:], fc[:, lo:hi], start=True,
                                 stop=True)
                nc.vector.tensor_copy(idxc[:, lo:hi], pr_v)

            def gather512(idx_ap, pool, tag):
                gt = pool.tile([128, GPC, ES], F16, tag=tag)
                nc.gpsimd.dma_gather(gt[:], tbl_ap, idx_ap, VCHUNK, VCHUNK, ES)
                return gt

            def xlerp(gt, g, fx_ap):
                """[128 pts, 4 zy, 256 ch] = diff * fx + lo  (fp16, one op)."""
                dm = xp.tile([128, 4, C], F16, tag="dm")
                nc.vector.scalar_tensor_tensor(
                    dm[:].rearrange("p z c -> p (z c)"), gt[:, g, 1024:2048],
                    fx_ap, gt[:, g, 0:1024], op0=ALU.mult, op1=ALU.add)
                return dm

            def diag16(w4_ap, tag):
                """[128, GPC, 4, 128] diag tiles for a whole 512-pt sample:
                one broadcast multiply builds all 16 diagonals.
                w4_ap: [128, GPC, 4] fp16 zy-weights per point."""
                dg = dgp.tile([128, GPC, 4, 128], F16, tag=tag)
                nc.vector.tensor_tensor(
                    dg[:],
                    ident[:].rearrange("p (a b c) -> p a b c", a=1, b=1)
                    .to_broadcast([128, GPC, 4, 128]),
                    w4_ap.rearrange("p g (z u) -> p g z u", u=1)
                    .to_broadcast([128, GPC, 4, 128]),
                    op=ALU.mult)
                return dg

            def feat_transpose(dm, dg_g, tag="fsb", bufs=None):
                """8 accumulating diag-matmuls: blended featT in PSUM ->
                fp16 SBUF [128 ch-half, 2, 128 pts].
                dg_g: [128, 4, 128] diag tiles (one per zy) for this group."""
                pF = psf.tile([128, 2, 128], F32, space="PSUM", tag="pF")
                for h in range(2):
                    for zy in range(4):
                        nc.tensor.matmul(
                            pF[:, h, :], dm[:, zy, h * 128:(h + 1) * 128],
                            dg_g[:, zy, :], start=(zy == 0), stop=(zy == 3))
                fsb = ftp.tile([128, 2, 128], F16, tag=tag, bufs=bufs)
                nc.scalar.copy(fsb[:], pF[:])
                return fsb

            def feat_transpose16(gt, j, dgn_g, dgf_g):
                """PE-path blend: 16 accumulating diag-matmuls straight off
                the gathered tile (x-lerp folded in via the fx*w4 diags)."""
                pF = psf.tile([128, 2, 128], F32, space="PSUM", tag="pF")
                for hh in range(2):
                    for zy in range(4):
                        o = zy * 256 + hh * 128
                        nc.tensor.matmul(
                            pF[:, hh, :], gt[:, j, o:o + 128],
                            dgn_g[:, zy, :], start=(zy == 0), stop=False)
                        nc.tensor.matmul(
                            pF[:, hh, :], gt[:, j, 1024 + o:1024 + o + 128],
                            dgf_g[:, zy, :], start=False, stop=(zy == 3))
                fsb = ftp.tile([128, 2, 128], F16, tag="fsb")
                nc.scalar.copy(fsb[:], pF[:])
                return fsb

            def main_mm(out_ap, fsb, k, start, stop):
                nc.tensor.matmul(out_ap, fsb[:, 0, :], msum_a[:, k, :],
                                 start=start, stop=False)
                nc.tensor.matmul(out_ap, fsb[:, 1, :], msum_b[:, k, :],
                                 start=False, stop=stop)

            def centers_compute(vc):
                """Everything for chunk vc's centers except the matmuls that
                touch the output accumulator (deferred so this block can be
                emitted while the PREVIOUS chunk's neighbours are in flight):
                gather, x-lerp, featT, shift, neighbour coords, neighbour
                index math, idx round-trip."""
                h = {}
                gts_c = gather512(idxc[:, vc * 32:(vc + 1) * 32], cgp, "cgt")
                dgc = diag16(w4c[:, vc * GPC:(vc + 1) * GPC, :], "dgc")
                ncoord = ixp.tile([128, GPC, NN, 3], F32, tag="ncrd")
                h["fsb"] = []
                for g in range(GPC):
                    vt = vc * GPC + g
                    dm = xlerp(gts_c, g, frcC[:, vt, 0:1])
                    fsb = feat_transpose(dm, dgc[:, g], tag="cfsb", bufs=12)
                    h["fsb"].append(fsb)
                    # shift matmul -> [128 pts, 24]
                    pS = pss.tile([128, 3 * NN], F32, space="PSUM", tag="sh")
                    nc.tensor.matmul(pS[:], fsb[:, 0, :], wsh_a[:],
                                     start=True, stop=False)
                    nc.tensor.matmul(pS[:], fsb[:, 1, :], wsh_b[:],
                                     start=False, stop=True)
                    ssb = mp.tile([128, 3 * NN], F32, tag="ssb")
                    nc.scalar.copy(ssb[:], pS[:])
                    # neighbour coords: shift + b_shift + verts [128, NN, 3]
                    nc.vector.tensor_tensor(
                        ncoord[:, g, :, :],
                        ssb[:].rearrange("p (nn c) -> p nn c", c=3),
                        bsh[:], op=ALU.add)
                    nc.vector.tensor_tensor(
                        ncoord[:, g, :, :], ncoord[:, g, :, :],
                        verts[:, vt:vt + 1, :].to_broadcast([128, NN, 3]),
                        op=ALU.add)

                # ---- neighbour index math (whole chunk, S = 32) ----
                r16n = ixp.tile([128, GPC * NN], I16, tag="r16n")
                frcN = ixp.tile([128, GPC * NN, 3], F32, tag="frcN")
                w4n = ixp.tile([128, GPC * NN, 4], F16, tag="w4n")
                _emit_index_math(
                    nc, ixp, ncoord[:].rearrange("p g nn c -> p (g nn) c"),
                    GPC * NN, r16n[:], frcN, w4n, "n")
                h["frcN"], h["w4n"] = frcN, w4n
                # [coords; 1] for all 9 samples, grouped by g so that one
                # [36, 128] transpose + one K=36 matmul per group covers the
                # whole coord part of the contraction.
                ca4 = ixp.tile([128, GPC, 9, 4], F16, tag="ca4")
                nc.vector.tensor_copy(
                    ca4[:, :, 0, 0:3], verts[:, vc * GPC:(vc + 1) * GPC, :])
                nc.vector.tensor_copy(ca4[:, :, 1:9, 0:3], ncoord[:])
                nc.vector.memset(ca4[:, :, :, 3], 1.0)
                h["ct36"] = []
                for g in range(GPC):
                    pTn = pst.tile([36, 128], F16, space="PSUM", tag="pTn")
                    nc.tensor.transpose(
                        pTn[:], ca4[:, g, :, :].rearrange("p k c -> p (k c)"),
                        ident[:])
                    ct36 = mp.tile([36, 128], F16, tag="ct36", bufs=12)
                    nc.scalar.copy(ct36[:], pTn[:])
                    h["ct36"].append(ct36)

                # idx round-trip: DRAM layout "(g p nn)" keeps both DMAs
                # nn-contiguous; read back wrapped-16 + replicate once.
                scr_n = dpn.tile([VCHUNK * NN], I16, tag="scrn")
                nc.sync.dma_start(
                    scr_n[:].rearrange("(g p nn) -> p g nn", p=128, g=GPC),
                    r16n[:].rearrange("p (g nn) -> p g nn", nn=NN))
                t16n = ixp.tile([16, NN, VCHUNK // 16], I16, tag="t16n")
                nc.sync.dma_start(
                    t16n[:].rearrange("q nn (m1 m0) -> q nn m1 m0", m1=GPC),
                    scr_n[:].rearrange("(m1 m0 q nn) -> q nn m1 m0",
                                       m1=GPC, m0=8, q=16))
                h["t16n"] = t16n
                h["vc"] = vc
                return h

            def centers_finish(h):
                """idx conversion + replication; emitted a few neighbour
                blocks after centers_compute so the scratch round-trip
                latency never stalls the in-order DVE stream."""
                t16n = h["t16n"]
                fn = ixp.tile([16, NN, VCHUNK // 16], F32, tag="fn")
                nc.vector.tensor_copy(fn[:], t16n[:])
                pRn = psf.tile([128, 2, 128], F32, space="PSUM",
                               tag="pF", name=f"repn{h['vc']}")
                pRn_v = pRn[:].rearrange("p a b -> p (a b)")
                nc.tensor.matmul(
                    pRn_v, rep16[:], fn[:].rearrange("q nn m -> q (nn m)"),
                    start=True, stop=True)
                idxn = ixp.tile([128, NN, VCHUNK // 16], I16, tag="idxn")
                nc.vector.tensor_copy(
                    idxn[:].rearrange("p nn m -> p (nn m)"), pRn_v)
                h["idxn"] = idxn

            # 2-deep software pipeline: chunk vc's neighbour phase runs
            # while chunk vc+2's center phase (emitted inside it) covers the
            # idx-chain and gather latency.
            hs = {0: centers_compute(0)}
            if nchunk > 1:
                hs[1] = centers_compute(1)
            centers_finish(hs[0])
            if nchunk > 1:
                centers_finish(hs[1])
            for vc in range(nchunk):
                h = hs.pop(vc)
                # one full 2 KB bank per group: psum allows only one pending
                # accumulation group per bank ("zero region")
                out4 = pso.tile([128, GPC, 2 * C], F32, space="PSUM",
                                tag="out", name=f"out{vc}")
                out_ps = [out4[:, g, 0:C] for g in range(GPC)]
                # deferred center + coord matmuls for this chunk
                for g in range(GPC):
                    main_mm(out_ps[g], h["fsb"][g], 0, start=True, stop=False)
                    nc.tensor.matmul(out_ps[g], h["ct36"][g][:], msum_c[:],
                                     start=False, stop=False)
                idxn, frcN, w4n = h["idxn"], h["frcN"], h["w4n"]
                w4n_v = w4n[:].rearrange("p (g nn) z -> p g nn z", nn=NN)
                # fx * w4 for the PE-path samples (one small op per chunk)
                fw4n = ixp.tile([128, GPC * NN, 4], F16, tag="fw4n")
                nc.vector.tensor_tensor(
                    fw4n[:], w4n[:],
                    frcN[:, :, 0:1].to_broadcast([128, GPC * NN, 4]),
                    op=ALU.mult)
                fw4n_v = fw4n[:].rearrange("p (g nn) z -> p g nn z", nn=NN)

                # ---- neighbours: gathers paired (1024 idx) to halve the
                # SWDGE fixed prep cost ----
                for pr in range(NN // 2):
                    gtn = gp.tile([128, 2 * GPC, ES], F16, tag="ngt")
                    nc.gpsimd.dma_gather(
                        gtn[:], tbl_ap, idxn[:, 2 * pr:2 * pr + 2, :],
                        2 * VCHUNK, 2 * VCHUNK, ES)
                    for sub in range(2):
                        nn_i = 2 * pr + sub
                        dgn = diag16(w4n_v[:, :, nn_i, :], "dgn")
                        if nn_i in (3, 5, 7):
                            # PE-path: x-lerp folds into 8 extra diag-mms
                            dgf = diag16(fw4n_v[:, :, nn_i, :], "dgf")
                            for g in range(GPC):
                                fsb = feat_transpose16(
                                    gtn, 4 * sub + g, dgn[:, g], dgf[:, g])
                                main_mm(out_ps[g], fsb, nn_i + 1, start=False,
                                        stop=(nn_i == NN - 1))
                        else:
                            for g in range(GPC):
                                col = g * NN + nn_i
                                dm = xlerp(gtn, 4 * sub + g, frcN[:, col, 0:1])
                                fsb = feat_transpose(dm, dgn[:, g])
                                main_mm(out_ps[g], fsb, nn_i + 1, start=False,
                                        stop=(nn_i == NN - 1))
                    if pr == 0 and vc + 2 < nchunk:
                        hs[vc + 2] = centers_compute(vc + 2)
                    if pr == 2 and vc + 2 < nchunk:
                        centers_finish(hs[vc + 2])

                # ---- epilogue ----
                for g in range(GPC):
                    osb = mp.tile([128, C], F32, tag="osb")
                    nc.scalar.copy(osb[:], out_ps[g])
                    lo = (vc * GPC + g) * 128
                    nc.sync.dma_start(out_d[lo:lo + 128, :], osb[:])

    nc.compile()
    return nc


# --------------------------------------------------------------- host wrapper
_CACHED = {}


def _block_index():
    """Static [V] row indices for the 8 block entries (zy lo + x+1)."""
    if "bidx" in _CACHED:
        return _CACHED["bidx"]
    z, y, x = np.meshgrid(np.arange(GRID), np.arange(GRID), np.arange(GRID),
                          indexing="ij")
    x1 = np.minimum(x + 1, GRID - 1)
    lo, hi = [], []
    for dz in (0, 1):
        for dy in (0, 1):
            zc = np.minimum(z + dz, GRID - 1)
            yc = np.minimum(y + dy, GRID - 1)
            lo.append(((zc * GRID + yc) * GRID + x).ravel())
            hi.append(((zc * GRID + yc) * GRID + x1).ravel())
    bidx = (np.stack(lo, 1).astype(np.int32), np.stack(hi, 1).astype(np.int32))
    _CACHED["bidx"] = bidx
    return bidx


def _host_prep(x, W_shift, b_shift, W_diff, b_diff, W_center, b_center,
               W_sum, b_sum):
    lo_i, hi_i = _block_index()
    tables = np.empty((B, V, 8, C), np.float16)
    for b in range(B):
        xt = np.ascontiguousarray(x[b].reshape(C, V).T)     # [V, C] f32
        lo = xt[lo_i]                                        # [V, 4, C]
        tables[b, :, 0:4, :] = lo
        tables[b, :, 4:8, :] = xt[hi_i] - lo
    tables = tables.reshape(B, V * ES)

    M = np.einsum("ock,cd->okd", W_sum.astype(np.float64),
                  W_diff.astype(np.float64))                 # [256, 9, 259]
    M = np.transpose(M, (1, 0, 2)).copy()                    # [9, 256, 259]
    M[0] += W_center.astype(np.float64)
    bias = (W_sum.astype(np.float64).sum(-1) @ b_diff.astype(np.float64)
            + b_sum + b_center)                              # [256]
    msum = np.zeros((9, C + 4, C), np.float16)
    for k in range(9):
        msum[k, :C + 3, :] = M[k].T.astype(np.float16)
    msum[0, C + 3, :] = bias.astype(np.float16)
    msum_a = np.ascontiguousarray(np.transpose(msum[:, 0:128, :], (1, 0, 2)))
    msum_b = np.ascontiguousarray(np.transpose(msum[:, 128:256, :], (1, 0, 2)))
    # [36, 256]: row k*4+j = coord row j (x, y, z, bias) of M_k
    msum_c = np.ascontiguousarray(
        msum[:, 256:260, :].reshape(36, C))

    wsh = W_shift.T.astype(np.float16)                       # [256, 24]
    bsh = np.broadcast_to(
        b_shift.astype(np.float32).reshape(NN, 3), (128, NN, 3)).copy()
    return (tables, msum_a, msum_b, msum_c,
            np.ascontiguousarray(wsh[0:128]),
            np.ascontiguousarray(wsh[128:256]), bsh)


def kernel(x, vertices, W_shift, b_shift, W_diff, b_diff, W_center, b_center,
           W_sum, b_sum):
    if "nc" not in _CACHED:
        _CACHED["nc"] = build_program()
    nc = _CACHED["nc"]

    tables, msum_a, msum_b, msum_c, wsh_a, wsh_b, bsh = _host_prep(
        x, W_shift, b_shift, W_diff, b_diff, W_center, b_center, W_sum, b_sum)

    in_maps = []
    for core in range(8):
        b, h = divmod(core, 2)
        in_maps.append({
            "verts": np.ascontiguousarray(
                vertices[b, h * NVC:(h + 1) * NVC]).astype(np.float32),
            "table": tables[b],
            "msum_a": msum_a, "msum_b": msum_b, "msum_c": msum_c,
            "wsh_a": wsh_a, "wsh_b": wsh_b, "bsh": bsh,
            "rep16": np.tile(np.eye(16, dtype=np.float32), 8),
        })

    res = run_bass_kernel_spmd(nc, in_maps, core_ids=list(range(8)))
    out = np.empty((B, N, C), np.float32)
    for core in range(8):
        b, h = divmod(core, 2)
        out[b, h * NVC:(h + 1) * NVC] = res.results[core]["out"]
    return out


# revision 33
# speedup vs baseline: 1.1378x; 1.1340x over previous
"""Trainium2 Bass kernel for nn_SamplingBlock (gnn_message_passing).

Strategy
--------
8 cores = (batch b in 0..3) x (vertex half h in 0..1); each core owns 4096
vertices of one batch, fully data-parallel (no collectives).

Host-side weight folding (weights-only algebra):
    M_k   = W_sum[:,:,k] @ W_diff          (k = 0..8; [256, 259])
    M_0  += W_center
    bias  = sum_k W_sum[:,:,k] @ b_diff + b_sum + b_center       ([256])
    out[n] = sum_k M_k @ [feat_k; coords_k; 1]  (the 1-row carries the bias)

Host-side volume re-layout (fp16 "block table"):
    For every voxel r = z*1024 + y*32 + x the table stores the full 2x2x2
    neighbourhood as one contiguous 4 KB element of 8 rows x 256 ch:
      rows 0..3:  lo_zy  = vol[z+dz, y+dy, x]        (zy = dz*2+dy)
      rows 4..7:  d_zy   = vol[z+dz, y+dy, x+1] - lo (x+1 clamped)
    One dma_gather per 512-point sample fetches everything trilinear needs.

Device pipeline per 512-pt sample (Tile framework):
  1. x-lerp on DVE at packed rates: dm = d*fx (tensor_scalar, 4x mode)
     + lo (tensor_tensor, 2x mode)  -> [128 pts, 4 zy, 256 ch] fp16
  2. y/z-lerp folded into the PE transposes: 8 accumulating matmuls with
     rhs = diag(w_zy) produce the blended AND transposed features directly
     in PSUM:  featT[ch, pt] = sum_zy dm[pt, zy, ch] * w_zy[pt]
  3. main matmul: featT (fp16) x folded weights, PSUM accumulation over
     k = 0..8 (center + 8 neighbours) -> out [pts, 256]
Center samples additionally run the shift matmul -> neighbour coords ->
neighbour index math -> batched idx DMA round-trip -> 8 neighbour gathers.
Center gathers are prefetched one chunk ahead to keep DMA busy.
"""

import os
import sys

import numpy as np

for _p in ("/opt/trn_rl_repo", "/root/.axon_site/_ro/trn_rl_repo"):
    if os.path.isdir(_p) and _p not in sys.path:
        sys.path.insert(0, _p)
        break

import concourse.bacc as bacc
import concourse.bass as bass
import concourse.mybir as mybir
import concourse.tile as tile
from concourse.bass_utils import run_bass_kernel_spmd
from concourse.masks import make_identity

# ---------------------------------------------------------------- constants
B, N, C, NN = 4, 8192, 256, 8
GRID = 32
V = GRID * GRID * GRID             # 32768 rows
NVC = N // 2                       # vertices per core = 4096
VCHUNK = 512                       # vertices per chunk
GPC = VCHUNK // 128                # 128-pt groups per chunk = 4
ES = 8 * C                         # gather element: 8 rows x 256 ch (fp16)
F32 = mybir.dt.float32
F16 = mybir.dt.float16
I16 = mybir.dt.int16
I32 = mybir.dt.int32
ALU = mybir.AluOpType


# ------------------------------------------------------------- device program
def _emit_index_math(nc, sb, coords, S, r16_out, frc, w4, pfx):
    """coords: [128, S, 3] f32 AP (x, y, z normalized, unclipped).
    Writes r16_out [128, S] int16 row indices, frc [128, S, 3] f32
    fractions (frc[..,0] = fx) and w4 [128, S, 4] f32 zy corner weights
    ordered j = dz*2 + dy."""
    g = sb.tile([128, S, 3], F32, tag=pfx + "ixg")
    nc.vector.tensor_scalar(g[:], coords, 15.5, 15.5, op0=ALU.mult, op1=ALU.add)
    nc.vector.tensor_scalar(g[:], g[:], float(GRID - 1), 0.0, op0=ALU.min,
                            op1=ALU.max)
    # floor(g) robust to f32->int rounding mode: q = int(g); q -= (g < q)
    qi = sb.tile([128, S, 3], I32, tag=pfx + "ixq")
    nc.vector.tensor_copy(qi[:], g[:])
    i0 = sb.tile([128, S, 3], F32, tag=pfx + "ixi")
    nc.vector.tensor_copy(i0[:], qi[:])
    nc.vector.tensor_tensor(frc[:], g[:], i0[:], op=ALU.subtract)
    msk = sb.tile([128, S, 3], F32, tag=pfx + "ixm")
    nc.vector.tensor_scalar(msk[:], frc[:], 0.0, None, op0=ALU.is_lt)
    nc.vector.tensor_tensor(i0[:], i0[:], msk[:], op=ALU.subtract)
    nc.vector.tensor_tensor(frc[:], g[:], i0[:], op=ALU.subtract)
    # r00 = z*1024 + y*32 + x  (exact in f32)
    r = sb.tile([128, S], F32, tag=pfx + "ixr")
    nc.vector.tensor_scalar(r[:], i0[:, :, 2], 1024.0, None, op0=ALU.mult)
    t = sb.tile([128, S], F32, tag=pfx + "ixt")
    nc.vector.tensor_scalar(t[:], i0[:, :, 1], 32.0, None, op0=ALU.mult)
    nc.vector.tensor_tensor(r[:], r[:], t[:], op=ALU.add)
    nc.vector.tensor_tensor(r[:], r[:], i0[:, :, 0], op=ALU.add)
    nc.vector.tensor_copy(r16_out, r[:])
    inv = sb.tile([128, S, 3], F32, tag=pfx + "ixv")
    nc.vector.tensor_scalar(inv[:], frc[:], -1.0, 1.0, op0=ALU.mult, op1=ALU.add)
    # w4[j = dz*2+dy]: (dy ? fy : 1-fy) * (dz ? fz : 1-fz)
    for j, (ys, zs) in enumerate(((inv, inv), (frc, inv), (inv, frc),
                                  (frc, frc))):
        nc.vector.tensor_tensor(w4[:, :, j], ys[:, :, 1], zs[:, :, 2],
                                op=ALU.mult)


def build_program(nvc=NVC):
    nchunk = nvc // VCHUNK
    nc = bacc.Bacc("TRN2", target_bir_lowering=False, debug=False)

    verts_d = nc.dram_tensor("verts", [nvc, 3], F32, kind="ExternalInput")
    table_d = nc.dram_tensor("table", [V * ES], F16, kind="ExternalInput")
    msum_a_d = nc.dram_tensor("msum_a", [128, 9, C], F16, kind="ExternalInput")
    msum_b_d = nc.dram_tensor("msum_b", [128, 9, C], F16, kind="ExternalInput")
    msum_c_d = nc.dram_tensor("msum_c", [36, C], F16, kind="ExternalInput")
    wsh_a_d = nc.dram_tensor("wsh_a", [128, 3 * NN], F16, kind="ExternalInput")
    wsh_b_d = nc.dram_tensor("wsh_b", [128, 3 * NN], F16, kind="ExternalInput")
    bsh_d = nc.dram_tensor("bsh", [128, NN, 3], F32, kind="ExternalInput")
    rep16_d = nc.dram_tensor("rep16", [16, 128], F32, kind="ExternalInput")
    out_d = nc.dram_tensor("out", [nvc, C], F32, kind="ExternalOutput")

    tbl_ap = bass.AP(table_d, 0, [[ES, V], [1, ES]])
    SC = nvc // 128                     # center cols per partition

    with tile.TileContext(nc) as tc:
        with (
            tc.tile_pool(name="cst", bufs=1) as cst,
            tc.tile_pool(name="wp", bufs=1) as wp,
            tc.tile_pool(name="ix", bufs=3) as ixp,
            tc.tile_pool(name="cg", bufs=3) as cgp,
            tc.tile_pool(name="ng", bufs=2) as gp,
            tc.tile_pool(name="xl", bufs=3) as xp,
            tc.tile_pool(name="dg", bufs=2) as dgp,
            tc.tile_pool(name="ft", bufs=3) as ftp,
            tc.tile_pool(name="mi", bufs=2) as mp,
            tc.tile_pool(name="drc", bufs=1, space="DRAM") as dpc,
            tc.tile_pool(name="drn", bufs=3, space="DRAM") as dpn,
            tc.tile_pool(name="pso", bufs=1, space="PSUM") as pso,
            tc.tile_pool(name="psf", bufs=2, space="PSUM") as psf,
            tc.tile_pool(name="pss", bufs=1, space="PSUM") as pss,
            tc.tile_pool(name="pst", bufs=1, space="PSUM") as pst,
        ):
            ident = cst.tile([128, 128], F16)
            make_identity(nc, ident[:])
            msum_a = cst.tile([128, 9, C], F16)
            msum_b = cst.tile([128, 9, C], F16)
            msum_c = cst.tile([36, C], F16)
            wsh_a = cst.tile([128, 3 * NN], F16)
            wsh_b = cst.tile([128, 3 * NN], F16)
            bsh = cst.tile([128, NN, 3], F32)
            rep16 = cst.tile([16, 128], F32)
            nc.sync.dma_start(msum_a[:], msum_a_d[:])
            nc.sync.dma_start(msum_b[:], msum_b_d[:])
            nc.sync.dma_start(msum_c[:], msum_c_d[:])
            nc.sync.dma_start(wsh_a[:], wsh_a_d[:])
            nc.sync.dma_start(wsh_b[:], wsh_b_d[:])
            nc.sync.dma_start(bsh[:], bsh_d[:])
            nc.sync.dma_start(rep16[:], rep16_d[:])

            verts = cst.tile([128, SC, 3], F32)
            nc.sync.dma_start(
                verts[:], verts_d[:].rearrange("(vt p) c -> p vt c", p=128))

            # ---- whole-core center index math + coords4 ----
            r16c = wp.tile([128, SC], I16)
            frcC = wp.tile([128, SC, 3], F32)
            w4c = wp.tile([128, SC, 4], F16)
            _emit_index_math(nc, wp, verts[:], SC, r16c[:], frcC, w4c, "c")

            # center indices: DRAM round-trip into wrapped-16 layout, then
            # replicate to 128 partitions via the rep16 matmul -- once for
            # the whole core.
            scr_c = dpc.tile([nvc], I16)
            nc.sync.dma_start(
                scr_c[:].rearrange("(vt p) -> p vt", p=128), r16c[:])
            t16c = wp.tile([16, nvc // 16], I16)
            nc.sync.dma_start(
                t16c[:], scr_c[:].rearrange("(m q) -> q m", q=16))
            fc = wp.tile([16, nvc // 16], F32)
            nc.vector.tensor_copy(fc[:], t16c[:])
            idxc = wp.tile([128, nvc // 16], I16)
            for half in range(max(1, nvc // 4096)):
                lo, hi = half * 256, min((half + 1) * 256, nvc // 16)
                pr = psf.tile([128, 2, 128], F32, space="PSUM", tag="pF",
                              name=f"repc{half}")
                pr_v = pr[:].rearrange("p a b -> p (a b)")[:, 0:hi - lo]
                nc.tensor.matmul(pr_v, rep16[:], fc[:, lo:hi], start=True,
                                 stop=True)
                nc.vector.tensor_copy(idxc[:, lo:hi], pr_v)

            def gather512(idx_ap, pool, tag):
                gt = pool.tile([128, GPC, ES], F16, tag=tag)
                nc.gpsimd.dma_gather(gt[:], tbl_ap, idx_ap, VCHUNK, VCHUNK, ES)
                return gt

            def xlerp(gt, g, fx_ap):
                """[128 pts, 4 zy, 256 ch] = diff * fx + lo  (fp16, one op)."""
                dm = xp.tile([128, 4, C], F16, tag="dm")
                nc.vector.scalar_tensor_tensor(
                    dm[:].rearrange("p z c -> p (z c)"), gt[:, g, 1024:2048],
                    fx_ap, gt[:, g, 0:1024], op0=ALU.mult, op1=ALU.add)
                return dm

            def diag16(w4_ap, tag):
                """[128, GPC, 4, 128] diag tiles for a whole 512-pt sample:
                one broadcast multiply builds all 16 diagonals.
                w4_ap: [128, GPC, 4] fp16 zy-weights per point."""
                dg = dgp.tile([128, GPC, 4, 128], F16, tag=tag)
                nc.vector.tensor_tensor(
                    dg[:],
                    ident[:].rearrange("p (a b c) -> p a b c", a=1, b=1)
                    .to_broadcast([128, GPC, 4, 128]),
                    w4_ap.rearrange("p g (z u) -> p g z u", u=1)
                    .to_broadcast([128, GPC, 4, 128]),
                    op=ALU.mult)
                return dg

            def feat_transpose(dm, dg_g, tag="fsb", bufs=None):
                """8 accumulating diag-matmuls: blended featT in PSUM ->
                fp16 SBUF [128 ch-half, 2, 128 pts].
                dg_g: [128, 4, 128] diag tiles (one per zy) for this group."""
                pF = psf.tile([128, 2, 128], F32, space="PSUM", tag="pF")
                for h in range(2):
                    for zy in range(4):
                        nc.tensor.matmul(
                            pF[:, h, :], dm[:, zy, h * 128:(h + 1) * 128],
                            dg_g[:, zy, :], start=(zy == 0), stop=(zy == 3))
                fsb = ftp.tile([128, 2, 128], F16, tag=tag, bufs=bufs)
                nc.scalar.copy(fsb[:], pF[:])
                return fsb

            def feat_transpose16(gt, j, dgn_g, dgf_g):
                """PE-path blend: 16 accumulating diag-matmuls straight off
                the gathered tile (x-lerp folded in via the fx*w4 diags)."""
                pF = psf.tile([128, 2, 128], F32, space="PSUM", tag="pF")
                for hh in range(2):
                    for zy in range(4):
                        o = zy * 256 + hh * 128
                        nc.tensor.matmul(
                            pF[:, hh, :], gt[:, j, o:o + 128],
                            dgn_g[:, zy, :], start=(zy == 0), stop=False)
                        nc.tensor.matmul(
                            pF[:, hh, :], gt[:, j, 1024 + o:1024 + o + 128],
                            dgf_g[:, zy, :], start=False, stop=(zy == 3))
                fsb = ftp.tile([128, 2, 128], F16, tag="fsb")
                nc.scalar.copy(fsb[:], pF[:])
                return fsb

            def main_mm(out_ap, fsb, k, start, stop):
                nc.tensor.matmul(out_ap, fsb[:, 0, :], msum_a[:, k, :],
                                 start=start, stop=False)
                nc.tensor.matmul(out_ap, fsb[:, 1, :], msum_b[:, k, :],
                                 start=False, stop=stop)

            def centers_compute(vc):
                """Everything for chunk vc's centers except the matmuls that
                touch the output accumulator (deferred so this block can be
                emitted while the PREVIOUS chunk's neighbours are in flight):
                gather, x-lerp, featT, shift, neighbour coords, neighbour
                index math, idx round-trip."""
                h = {}
                gts_c = cg_pending.pop(vc)
                dgc = diag16(w4c[:, vc * GPC:(vc + 1) * GPC, :], "dgc")
                ncoord = ixp.tile([128, GPC, NN, 3], F32, tag="ncrd")
                h["fsb"] = []
                for g in range(GPC):
                    vt = vc * GPC + g
                    dm = xlerp(gts_c, g, frcC[:, vt, 0:1])
                    fsb = feat_transpose(dm, dgc[:, g], tag="cfsb", bufs=12)
                    h["fsb"].append(fsb)
                    # shift matmul -> [128 pts, 24]
                    pS = pss.tile([128, 3 * NN], F32, space="PSUM", tag="sh")
                    nc.tensor.matmul(pS[:], fsb[:, 0, :], wsh_a[:],
                                     start=True, stop=False)
                    nc.tensor.matmul(pS[:], fsb[:, 1, :], wsh_b[:],
                                     start=False, stop=True)
                    ssb = mp.tile([128, 3 * NN], F32, tag="ssb")
                    nc.scalar.copy(ssb[:], pS[:])
                    # neighbour coords: shift + b_shift + verts [128, NN, 3]
                    nc.vector.tensor_tensor(
                        ncoord[:, g, :, :],
                        ssb[:].rearrange("p (nn c) -> p nn c", c=3),
                        bsh[:], op=ALU.add)
                    nc.vector.tensor_tensor(
                        ncoord[:, g, :, :], ncoord[:, g, :, :],
                        verts[:, vt:vt + 1, :].to_broadcast([128, NN, 3]),
                        op=ALU.add)

                # ---- neighbour index math (whole chunk, S = 32) ----
                r16n = ixp.tile([128, GPC * NN], I16, tag="r16n")
                frcN = ixp.tile([128, GPC * NN, 3], F32, tag="frcN")
                w4n = ixp.tile([128, GPC * NN, 4], F16, tag="w4n")
                _emit_index_math(
                    nc, ixp, ncoord[:].rearrange("p g nn c -> p (g nn) c"),
                    GPC * NN, r16n[:], frcN, w4n, "n")
                h["frcN"], h["w4n"] = frcN, w4n
                # [coords; 1] for all 9 samples, grouped by g so that one
                # [36, 128] transpose + one K=36 matmul per group covers the
                # whole coord part of the contraction.
                ca4 = ixp.tile([128, GPC, 9, 4], F16, tag="ca4")
                nc.vector.tensor_copy(
                    ca4[:, :, 0, 0:3], verts[:, vc * GPC:(vc + 1) * GPC, :])
                nc.vector.tensor_copy(ca4[:, :, 1:9, 0:3], ncoord[:])
                nc.vector.memset(ca4[:, :, :, 3], 1.0)
                h["ct36"] = []
                for g in range(GPC):
                    pTn = pst.tile([36, 128], F16, space="PSUM", tag="pTn")
                    nc.tensor.transpose(
                        pTn[:], ca4[:, g, :, :].rearrange("p k c -> p (k c)"),
                        ident[:])
                    ct36 = mp.tile([36, 128], F16, tag="ct36", bufs=12)
                    nc.scalar.copy(ct36[:], pTn[:])
                    h["ct36"].append(ct36)

                # idx round-trip: DRAM layout "(g p nn)" keeps both DMAs
                # nn-contiguous; read back wrapped-16 + replicate once.
                scr_n = dpn.tile([VCHUNK * NN], I16, tag="scrn")
                nc.sync.dma_start(
                    scr_n[:].rearrange("(g p nn) -> p g nn", p=128, g=GPC),
                    r16n[:].rearrange("p (g nn) -> p g nn", nn=NN))
                t16n = ixp.tile([16, NN, VCHUNK // 16], I16, tag="t16n")
                nc.sync.dma_start(
                    t16n[:].rearrange("q nn (m1 m0) -> q nn m1 m0", m1=GPC),
                    scr_n[:].rearrange("(m1 m0 q nn) -> q nn m1 m0",
                                       m1=GPC, m0=8, q=16))
                h["t16n"] = t16n
                h["vc"] = vc
                return h

            def centers_finish(h):
                """idx conversion + replication; emitted a few neighbour
                blocks after centers_compute so the scratch round-trip
                latency never stalls the in-order DVE stream."""
                t16n = h["t16n"]
                fn = ixp.tile([16, NN, VCHUNK // 16], F32, tag="fn")
                nc.vector.tensor_copy(fn[:], t16n[:])
                pRn = psf.tile([128, 2, 128], F32, space="PSUM",
                               tag="pF", name=f"repn{h['vc']}")
                pRn_v = pRn[:].rearrange("p a b -> p (a b)")
                nc.tensor.matmul(
                    pRn_v, rep16[:], fn[:].rearrange("q nn m -> q (nn m)"),
                    start=True, stop=True)
                idxn = ixp.tile([128, NN, VCHUNK // 16], I16, tag="idxn")
                nc.vector.tensor_copy(
                    idxn[:].rearrange("p nn m -> p (nn m)"), pRn_v)
                h["idxn"] = idxn

            # 2-deep software pipeline: chunk vc's neighbour phase runs
            # while chunk vc+2's center phase (emitted inside it) covers the
            # idx-chain and gather latency.
            cg_pending = {}

            def cg_issue(vc):
                if vc < nchunk:
                    cg_pending[vc] = gather512(
                        idxc[:, vc * 32:(vc + 1) * 32], cgp, "cgt")

            cg_issue(0)
            cg_issue(1)
            hs = {0: centers_compute(0)}
            if nchunk > 1:
                hs[1] = centers_compute(1)
            centers_finish(hs[0])
            if nchunk > 1:
                centers_finish(hs[1])
            for vc in range(nchunk):
                h = hs.pop(vc)
                # one full 2 KB bank per group: psum allows only one pending
                # accumulation group per bank ("zero region")
                out4 = pso.tile([128, GPC, 2 * C], F32, space="PSUM",
                                tag="out", name=f"out{vc}")
                out_ps = [out4[:, g, 0:C] for g in range(GPC)]
                # deferred center + coord matmuls for this chunk
                for g in range(GPC):
                    main_mm(out_ps[g], h["fsb"][g], 0, start=True, stop=False)
                    nc.tensor.matmul(out_ps[g], h["ct36"][g][:], msum_c[:],
                                     start=False, stop=False)
                idxn, frcN, w4n = h["idxn"], h["frcN"], h["w4n"]
                w4n_v = w4n[:].rearrange("p (g nn) z -> p g nn z", nn=NN)

                # issue chunk vc+2's center gather ahead of this chunk's
                # neighbour gathers so its data lands before the (soon to be
                # emitted) centers_compute(vc+2) x-lerps need it
                cg_issue(vc + 2)
                # ---- neighbours: gathers paired (1024 idx) to halve the
                # SWDGE fixed prep cost ----
                for pr in range(NN // 2):
                    gtn = gp.tile([128, 2 * GPC, ES], F16, tag="ngt")
                    nc.gpsimd.dma_gather(
                        gtn[:], tbl_ap, idxn[:, 2 * pr:2 * pr + 2, :],
                        2 * VCHUNK, 2 * VCHUNK, ES)
                    for sub in range(2):
                        nn_i = 2 * pr + sub
                        dgn = diag16(w4n_v[:, :, nn_i, :], "dgn")
                        for g in range(GPC):
                            col = g * NN + nn_i
                            dm = xlerp(gtn, 4 * sub + g, frcN[:, col, 0:1])
                            fsb = feat_transpose(dm, dgn[:, g])
                            main_mm(out_ps[g], fsb, nn_i + 1, start=False,
                                    stop=(nn_i == NN - 1))
                    if pr == 0 and vc + 2 < nchunk:
                        hs[vc + 2] = centers_compute(vc + 2)
                    if pr == 2 and vc + 2 < nchunk:
                        centers_finish(hs[vc + 2])

                # ---- epilogue ----
                for g in range(GPC):
                    osb = mp.tile([128, C], F32, tag="osb")
                    nc.scalar.copy(osb[:], out_ps[g])
                    lo = (vc * GPC + g) * 128
                    nc.sync.dma_start(out_d[lo:lo + 128, :], osb[:])

    nc.compile()
    return nc


# --------------------------------------------------------------- host wrapper
_CACHED = {}


def _block_index():
    """Static [V] row indices for the 8 block entries (zy lo + x+1)."""
    if "bidx" in _CACHED:
        return _CACHED["bidx"]
    z, y, x = np.meshgrid(np.arange(GRID), np.arange(GRID), np.arange(GRID),
                          indexing="ij")
    x1 = np.minimum(x + 1, GRID - 1)
    lo, hi = [], []
    for dz in (0, 1):
        for dy in (0, 1):
            zc = np.minimum(z + dz, GRID - 1)
            yc = np.minimum(y + dy, GRID - 1)
            lo.append(((zc * GRID + yc) * GRID + x).ravel())
            hi.append(((zc * GRID + yc) * GRID + x1).ravel())
    bidx = (np.stack(lo, 1).astype(np.int32), np.stack(hi, 1).astype(np.int32))
    _CACHED["bidx"] = bidx
    return bidx


def _host_prep(x, W_shift, b_shift, W_diff, b_diff, W_center, b_center,
               W_sum, b_sum):
    lo_i, hi_i = _block_index()
    tables = np.empty((B, V, 8, C), np.float16)
    for b in range(B):
        xt = np.ascontiguousarray(x[b].reshape(C, V).T)     # [V, C] f32
        lo = xt[lo_i]                                        # [V, 4, C]
        tables[b, :, 0:4, :] = lo
        tables[b, :, 4:8, :] = xt[hi_i] - lo
    tables = tables.reshape(B, V * ES)

    M = np.einsum("ock,cd->okd", W_sum.astype(np.float64),
                  W_diff.astype(np.float64))                 # [256, 9, 259]
    M = np.transpose(M, (1, 0, 2)).copy()                    # [9, 256, 259]
    M[0] += W_center.astype(np.float64)
    bias = (W_sum.astype(np.float64).sum(-1) @ b_diff.astype(np.float64)
            + b_sum + b_center)                              # [256]
    msum = np.zeros((9, C + 4, C), np.float16)
    for k in range(9):
        msum[k, :C + 3, :] = M[k].T.astype(np.float16)
    msum[0, C + 3, :] = bias.astype(np.float16)
    msum_a = np.ascontiguousarray(np.transpose(msum[:, 0:128, :], (1, 0, 2)))
    msum_b = np.ascontiguousarray(np.transpose(msum[:, 128:256, :], (1, 0, 2)))
    # [36, 256]: row k*4+j = coord row j (x, y, z, bias) of M_k
    msum_c = np.ascontiguousarray(
        msum[:, 256:260, :].reshape(36, C))

    wsh = W_shift.T.astype(np.float16)                       # [256, 24]
    bsh = np.broadcast_to(
        b_shift.astype(np.float32).reshape(NN, 3), (128, NN, 3)).copy()
    return (tables, msum_a, msum_b, msum_c,
            np.ascontiguousarray(wsh[0:128]),
            np.ascontiguousarray(wsh[128:256]), bsh)


def kernel(x, vertices, W_shift, b_shift, W_diff, b_diff, W_center, b_center,
           W_sum, b_sum):
    if "nc" not in _CACHED:
        _CACHED["nc"] = build_program()
    nc = _CACHED["nc"]

    tables, msum_a, msum_b, msum_c, wsh_a, wsh_b, bsh = _host_prep(
        x, W_shift, b_shift, W_diff, b_diff, W_center, b_center, W_sum, b_sum)

    in_maps = []
    for core in range(8):
        b, h = divmod(core, 2)
        in_maps.append({
            "verts": np.ascontiguousarray(
                vertices[b, h * NVC:(h + 1) * NVC]).astype(np.float32),
            "table": tables[b],
            "msum_a": msum_a, "msum_b": msum_b, "msum_c": msum_c,
            "wsh_a": wsh_a, "wsh_b": wsh_b, "bsh": bsh,
            "rep16": np.tile(np.eye(16, dtype=np.float32), 8),
        })

    res = run_bass_kernel_spmd(nc, in_maps, core_ids=list(range(8)))
    out = np.empty((B, N, C), np.float32)
    for core in range(8):
        b, h = divmod(core, 2)
        out[b, h * NVC:(h + 1) * NVC] = res.results[core]["out"]
    return out
